# revision 17
# baseline (speedup 1.0000x reference)
"""Trainium2 Bass kernel for the MiniBatchAUC pairwise surrogate loss.

Math: with s = sigmoid(logits), pos/neg the 0/1 target masks,
    loss_sum = sum_{i in P, j in N} (1 - s_i + s_j)^2
factorizes exactly (expand the square; the double sum separates):
    loss_sum = n_neg * Sp2 + 2 * Sp1 * Sn1 + n_pos * Sn2
      Sp1 = sum_P (1-s),  Sp2 = sum_P (1-s)^2,
      Sn1 = sum_N s,      Sn2 = sum_N s^2,
and with c = sum T, m1 = sum T*s, m2 = sum T*s^2, g1 = sum s, g2 = sum s^2:
      Sp1 = c - m1, Sp2 = c - 2*m1 + m2, Sn1 = g1 - m1, Sn2 = g2 - m2.
So the O(N^2) pairwise matrix is never materialized: each core reduces its
2048-element shard to 5 per-partition partial sums; the host all-reduces
the per-core partials and applies the closed form.

Per-core device program (SPMD, identical on all 8 cores), raw bacc with
manual semaphores (no TileContext exit drain). Critical-path layout
(everything else hides under the ~2.3us input-DMA latency):
  - SP: one HWDGE DMA in: x[128, 2, 16] bf16 = logits | targets (bf16 input
    halves the DMA payload to the 7ns/descriptor floor; since bf16 products
    accumulate exactly in the f32 accumulators below, end-to-end rel err
    stays ~9e-7).
  - Pool, early: memset the kv_writeback ctx index, then PREPARE the output
    DMA descriptors (kv_writeback prepare_only) so the ~1us SWDGE desc-gen
    runs during the input-DMA wait; the later trigger_dma pays only Pool SEQ
    + transfer + sem-prop instead of the full HWDGE issue chain.
  - DVE: c = sum(T) also hidden in the input wait.
  - ACT: s = sigmoid(L) IN PLACE over L. No accum_out: the accumulator read
    adds 187ns to the ACT->DVE handoff; g1 is a fused DVE op instead.
  - DVE: each moment is ONE fused scalar_tensor_tensor op
    (out = (in0 op0 scalar) op1 in1, accum_out = row-sum of out):
      m1: (T*1)*s -> ts      g1: (s*0)+s      g2: (s*1)*s
      m2: (ts*1)*ts          c:  (T*0)+T   (pre-sigmoid)
    g1 and c use plain tensor_scalar (mult, +accum), which keeps the DVE
    4x perf mode (~65ns); the scalar_tensor_tensor variant has no perf
    modes (77ns) but is still one op per moment. The f32 accum_out is
    exempt from the 2x dtype check, and bf16xbf16 products are exact in
    f32, so the sums carry only the input-quantization error (~9e-7 end
    to end). AluOpType.pow is rejected by the TensorScalarCacheReduce
    ISA check, so squares go through scalar_tensor_tensor.
    NOTE: tensor_tensor_reduce hard-crashes this runtime
    (NRT_EXEC_UNIT_UNRECOVERABLE); scalar_tensor_tensor is the fused
    multiply-reduce that works. Same-engine RAW (m2 reads m1's ts) carries
    no semaphore fence; the two interposed ops (g1, g2, ~130ns) cover the
    ~60ns DVE writeback latency -- validated bit-stable on hardware.
  - Pool: trigger_dma (with the V-wait attached to the trigger itself so
    its SEQ decode overlaps the wait) fires the prepared writeback of
    r [128,5] f32 -> o_dram.
No engine waits for the final DMA completion: the SWDGE queue drain is the
runtime's job; engines exit during the DMA-completion propagation window.
Host all-reduces the [8, 128, 5] partials and applies the closed form.
r columns: g2 | m1 | g1 | c | m2.
"""

import numpy as np

try:
    import concourse.bass as bass
except ImportError:  # concourse ships in the container, not on sys.path
    import sys

    sys.path.insert(0, "/opt/trn_rl_repo")
    import concourse.bass as bass

from concourse import bacc, mybir
from concourse import bass_utils

N = 16384
NCORES = 8
SHARD = N // NCORES  # 2048 elements per core
P = 128  # SBUF partitions
F = SHARD // P  # 16 free elements per partition

f32 = mybir.dt.float32
bf16 = mybir.dt.bfloat16
i32 = mybir.dt.int32

_CACHE: dict = {}


def _build():
    nc = bacc.Bacc(
        "TRN2",
        target_bir_lowering=False,
        debug=False,
        enable_asserts=False,
        num_devices=NCORES,
    )
    x_dram = nc.dram_tensor("x", [P, 2 * F], bf16, kind="ExternalInput").ap()
    # kv_writeback layout: out [batch=1, d_head_inner=128, d_head_outer=1,
    # n_ctx=5]; row-major this is bit-identical to [128, 5].
    o_dram = nc.dram_tensor("o", [1, P, 1, 5], f32, kind="ExternalOutput").ap()

    Sig = mybir.ActivationFunctionType.Sigmoid
    Mult = mybir.AluOpType.mult
    Add = mybir.AluOpType.add

    with (
        nc.sbuf_tensor([P, 2, F], bf16) as x,
        nc.sbuf_tensor([P, F], bf16) as ts,
        nc.sbuf_tensor([P, F], bf16) as scr0,
        nc.sbuf_tensor([P, F], bf16) as scr1,
        nc.sbuf_tensor([P, F], bf16) as scr2,
        nc.sbuf_tensor([P, F], bf16) as scr3,
        nc.sbuf_tensor([P, 1, 1, 5], f32) as r,  # g2 | m1 | g1 | c | m2
        nc.sbuf_tensor([P, 1], i32) as ctx_idx,
        nc.semaphore() as V,  # data chain: DMA +16, c +1, sigmoid +1, DVE +4
        nc.semaphore() as Q,  # pool chain: ctx memset, prep desc, out DMA
        nc.Block() as block,
    ):
        L = x[:, 0, :]  # becomes s after the in-place sigmoid
        T = x[:, 1, :]

        def rcol(k):
            return r[:, 0, 0, k : k + 1]

        def stt(out, in0, scalar, in1, op1, acc):
            return nc.vector.scalar_tensor_tensor(
                out, in0, scalar, in1, Mult, op1, accum_out=acc
            )

        def tsum(out, in0, acc):
            # plain tensor_scalar keeps its DVE 4x perf mode (the
            # scalar_tensor_tensor variant has none): ~65ns vs 77ns
            return nc.vector.tensor_scalar(
                out, in0, 1.0, 0.0, Mult, Add, accum_out=acc
            )

        @block.sync
        def _(sync):
            sync.dma_start(x[:], x_dram).then_inc(V, 16)

        @block.scalar
        def _(scalar):
            scalar.wait_ge(V, 16)
            nc.scalar.activation(L, L, Sig).then_inc(V, 1)  # in place: x=[s|T]

        @block.vector
        def _(vector):
            vector.wait_ge(V, 16)
            tsum(scr0[:], T, rcol(3)).then_inc(V, 1)  # c
            vector.wait_ge(V, 18)  # c and sigmoid both retired
            stt(ts[:], T, 1.0, L, Mult, rcol(1)).then_inc(V, 1)  # m1, ts=T*s
            tsum(scr1[:], L, rcol(2)).then_inc(V, 1)  # g1
            stt(scr2[:], L, 1.0, L, Mult, rcol(0)).then_inc(V, 1)  # g2
            # m2 reads ts (written 2 ops ago: ~140ns gap covers the ~60ns
            # DVE writeback latency; no fence needed)
            stt(scr3[:], ts[:], 1.0, ts[:], Mult, rcol(4)).then_inc(V, 1)  # m2

        @block.gpsimd
        def _(gpsimd):
            nc.gpsimd.memset(ctx_idx[:], 0).then_inc(Q, 1)
            gpsimd.wait_ge(Q, 1)  # ctx_idx valid before descriptor gen
            nc.gpsimd.kv_writeback(
                o_dram,
                r[:],
                ctx_idx[:],
                prepare_only=True,
                sem=Q,  # +16 when the triggered DMA lands
            ).then_inc(Q, 1)  # Q=2: descriptors written to the SWDGE ring
            gpsimd.wait_ge(Q, 2)
            # V>=22 (every moment retired in r) rides on the trigger itself:
            # the SEQ decode overlaps the wait, so the DMA fires ~60ns after
            # the last semaphore instead of after a separate EventSemaphore.
            nc.gpsimd.trigger_dma(count=1)._wait_ge(V, 22)

    nc.compile()
    return nc


def _get_nc():
    if "nc" not in _CACHE:
        _CACHE["nc"] = _build()
    return _CACHE["nc"]


def make_in_maps(logits: np.ndarray, targets: np.ndarray) -> list[dict]:
    import ml_dtypes

    bf = ml_dtypes.bfloat16
    lb = np.ascontiguousarray(logits, dtype=np.float32).astype(bf)
    tb = np.asarray(targets).astype(bf)  # values are 0/1; lossless in bf16
    in_maps = []
    for k in range(NCORES):
        sl = slice(k * SHARD, (k + 1) * SHARD)
        xk = np.empty((P, 2 * F), bf)
        xk[:, 0:F] = lb[sl].reshape(P, F)
        xk[:, F : 2 * F] = tb[sl].reshape(P, F)
        in_maps.append({"x": xk})
    return in_maps


def combine(outs: np.ndarray) -> np.ndarray:
    """All-reduce the [NCORES, P, 5] partials and apply the closed form."""
    tot = outs.astype(np.float64).sum(axis=(0, 1))
    g2, m1, g1, c, m2 = tot
    n_pos = c
    n_neg = float(N) - c
    sp1 = c - m1
    sp2 = c - 2.0 * m1 + m2
    sn1 = g1 - m1
    sn2 = g2 - m2
    loss = (n_neg * sp2 + 2.0 * sp1 * sn1 + n_pos * sn2) / (n_pos * n_neg)
    return np.array(loss, dtype=np.float32)


def kernel(logits: np.ndarray, targets: np.ndarray, **run_kwargs):
    nc = _get_nc()
    res = bass_utils.run_bass_kernel_spmd(
        nc, make_in_maps(logits, targets), core_ids=list(range(NCORES)), **run_kwargs
    )
    outs = np.stack([np.asarray(r["o"]).reshape(P, 5) for r in res.results])
    out = combine(outs)
    _CACHE["last_results"] = res
    return out


# revision 18
# speedup vs baseline: 1.0142x; 1.0142x over previous
"""Trainium2 Bass kernel for the MiniBatchAUC pairwise surrogate loss.

Math: with s = sigmoid(logits), pos/neg the 0/1 target masks,
    loss_sum = sum_{i in P, j in N} (1 - s_i + s_j)^2
factorizes exactly (expand the square; the double sum separates):
    loss_sum = n_neg * Sp2 + 2 * Sp1 * Sn1 + n_pos * Sn2
      Sp1 = sum_P (1-s),  Sp2 = sum_P (1-s)^2,
      Sn1 = sum_N s,      Sn2 = sum_N s^2,
and with c = sum T, m1 = sum T*s, m2 = sum T*s^2, g1 = sum s, g2 = sum s^2:
      Sp1 = c - m1, Sp2 = c - 2*m1 + m2, Sn1 = g1 - m1, Sn2 = g2 - m2.
So the O(N^2) pairwise matrix is never materialized: each core reduces its
2048-element shard to 5 per-partition partial sums; the host all-reduces
the per-core partials and applies the closed form.

Per-core device program (SPMD, identical on all 8 cores), raw bacc with
manual semaphores (no TileContext exit drain). Critical-path layout
(everything else hides under the ~2.3us input-DMA latency):
  - SP: one HWDGE DMA in: x[128, 2, 16] bf16 = logits | targets (bf16 input
    halves the DMA payload to the 7ns/descriptor floor; since bf16 products
    accumulate exactly in the f32 accumulators below, end-to-end rel err
    stays ~9e-7).
  - Pool, early: memset the kv_writeback ctx index, then PREPARE the output
    DMA descriptors (kv_writeback prepare_only) so the ~1us SWDGE desc-gen
    runs during the input-DMA wait; the later trigger_dma pays only Pool SEQ
    + transfer + sem-prop instead of the full HWDGE issue chain.
  - DVE: c = sum(T) also hidden in the input wait.
  - ACT: s = sigmoid(L) IN PLACE over L. No accum_out: the accumulator read
    adds 187ns to the ACT->DVE handoff; g1 is a fused DVE op instead.
  - DVE: each moment is ONE fused scalar_tensor_tensor op
    (out = (in0 op0 scalar) op1 in1, accum_out = row-sum of out):
      m1: (T*1)*s -> ts      g1: (s*0)+s      g2: (s*1)*s
      m2: (ts*1)*ts          c:  (T*0)+T   (pre-sigmoid)
    g1 and c use plain tensor_scalar (mult, +accum), which keeps the DVE
    4x perf mode (~65ns); the scalar_tensor_tensor variant has no perf
    modes (77ns) but is still one op per moment. The f32 accum_out is
    exempt from the 2x dtype check, and bf16xbf16 products are exact in
    f32, so the sums carry only the input-quantization error (~9e-7 end
    to end). AluOpType.pow is rejected by the TensorScalarCacheReduce
    ISA check, so squares go through scalar_tensor_tensor.
    NOTE: tensor_tensor_reduce hard-crashes this runtime
    (NRT_EXEC_UNIT_UNRECOVERABLE); scalar_tensor_tensor is the fused
    multiply-reduce that works. Same-engine RAW (m2 reads m1's ts) carries
    no semaphore fence; the two interposed ops (g1, g2, ~130ns) cover the
    ~60ns DVE writeback latency -- validated bit-stable on hardware.
  - Pool: trigger_dma (with the V-wait attached to the trigger itself so
    its SEQ decode overlaps the wait) fires the prepared writeback of
    r [128,5] f32 -> o_dram.
No engine waits for the final DMA completion: the SWDGE queue drain is the
runtime's job; engines exit during the DMA-completion propagation window.
Host all-reduces the [8, 128, 5] partials and applies the closed form.
r columns: g2 | m1 | g1 | c | m2.
"""

import numpy as np

try:
    import concourse.bass as bass
except ImportError:  # concourse ships in the container, not on sys.path
    import sys

    sys.path.insert(0, "/opt/trn_rl_repo")
    import concourse.bass as bass

from concourse import bacc, mybir
from concourse import bass_utils

N = 16384
NCORES = 8
SHARD = N // NCORES  # 2048 elements per core
P = 128  # SBUF partitions
F = SHARD // P  # 16 free elements per partition

f32 = mybir.dt.float32
bf16 = mybir.dt.bfloat16
i32 = mybir.dt.int32

_CACHE: dict = {}


def _build():
    nc = bacc.Bacc(
        "TRN2",
        target_bir_lowering=False,
        debug=False,
        enable_asserts=False,
        num_devices=NCORES,
    )
    x_dram = nc.dram_tensor("x", [P, 2 * F], bf16, kind="ExternalInput").ap()
    # kv_writeback layout: out [batch=1, d_head_inner=128, d_head_outer=1,
    # n_ctx=5]; row-major this is bit-identical to [128, 5].
    o_dram = nc.dram_tensor("o", [1, P, 1, 5], f32, kind="ExternalOutput").ap()

    Sig = mybir.ActivationFunctionType.Sigmoid
    Copy = mybir.ActivationFunctionType.Copy
    Mult = mybir.AluOpType.mult
    Add = mybir.AluOpType.add

    with (
        nc.sbuf_tensor([P, 2, F], bf16) as x,
        nc.sbuf_tensor([P, F], bf16) as ts,
        nc.sbuf_tensor([P, F], bf16) as scr0,
        nc.sbuf_tensor([P, F], bf16) as scr1,
        nc.sbuf_tensor([P, F], bf16) as scr2,
        nc.sbuf_tensor([P, F], bf16) as scr3,
        nc.sbuf_tensor([P, 1, 1, 5], f32) as r,  # g2 | m1 | g1 | c | m2
        nc.sbuf_tensor([P, 1], i32) as ctx_idx,
        nc.semaphore() as V,  # data chain: DMA +16, c +1, sigmoid +1, DVE +4
        nc.semaphore() as Q,  # pool chain: ctx memset, prep desc, out DMA
        nc.Block() as block,
    ):
        L = x[:, 0, :]  # becomes s after the in-place sigmoid
        T = x[:, 1, :]

        def rcol(k):
            return r[:, 0, 0, k : k + 1]

        def stt(out, in0, scalar, in1, op1, acc):
            return nc.vector.scalar_tensor_tensor(
                out, in0, scalar, in1, Mult, op1, accum_out=acc
            )

        def tsum(out, in0, acc):
            # plain tensor_scalar keeps its DVE 4x perf mode (the
            # scalar_tensor_tensor variant has none): ~65ns vs 77ns
            return nc.vector.tensor_scalar(
                out, in0, 1.0, 0.0, Mult, Add, accum_out=acc
            )

        @block.sync
        def _(sync):
            sync.dma_start(x[:], x_dram).then_inc(V, 16)

        @block.scalar
        def _(scalar):
            scalar.wait_ge(V, 16)
            nc.scalar.activation(L, L, Sig).then_inc(V, 1)  # in place: x=[s|T]
            # g1 via Copy+accum on the otherwise-idle ACT engine: lands at
            # sigmoid_end+385+211, just before DVE's 3-op chain retires.
            # Copy shares the Sigmoid ActFuncSet (single table load).
            nc.scalar.activation(scr1[:], L, Copy, accum_out=rcol(2)).then_inc(
                V, 1
            )  # g1

        @block.vector
        def _(vector):
            vector.wait_ge(V, 16)
            tsum(scr0[:], T, rcol(3)).then_inc(V, 1)  # c
            vector.wait_ge(V, 18)  # c and sigmoid both retired
            stt(ts[:], T, 1.0, L, Mult, rcol(1)).then_inc(V, 1)  # m1, ts=T*s
            stt(scr2[:], L, 1.0, L, Mult, rcol(0)).then_inc(V, 1)  # g2
            # m2 reads ts (written 1 op ago: g2's 77ns covers the ~60ns DVE
            # writeback latency; no fence needed -- same margin class as the
            # hardware-validated 74ns minimum)
            stt(scr3[:], ts[:], 1.0, ts[:], Mult, rcol(4)).then_inc(V, 1)  # m2

        @block.gpsimd
        def _(gpsimd):
            nc.gpsimd.memset(ctx_idx[:], 0).then_inc(Q, 1)
            gpsimd.wait_ge(Q, 1)  # ctx_idx valid before descriptor gen
            nc.gpsimd.kv_writeback(
                o_dram,
                r[:],
                ctx_idx[:],
                prepare_only=True,
                sem=Q,  # +16 when the triggered DMA lands
            ).then_inc(Q, 1)  # Q=2: descriptors written to the SWDGE ring
            gpsimd.wait_ge(Q, 2)
            # V>=22 (every moment retired in r) rides on the trigger itself:
            # the SEQ decode overlaps the wait, so the DMA fires ~60ns after
            # the last semaphore instead of after a separate EventSemaphore.
            nc.gpsimd.trigger_dma(count=1)._wait_ge(V, 22)

    nc.compile()
    return nc


def _get_nc():
    if "nc" not in _CACHE:
        _CACHE["nc"] = _build()
    return _CACHE["nc"]


def make_in_maps(logits: np.ndarray, targets: np.ndarray) -> list[dict]:
    import ml_dtypes

    bf = ml_dtypes.bfloat16
    lb = np.ascontiguousarray(logits, dtype=np.float32).astype(bf)
    tb = np.asarray(targets).astype(bf)  # values are 0/1; lossless in bf16
    in_maps = []
    for k in range(NCORES):
        sl = slice(k * SHARD, (k + 1) * SHARD)
        xk = np.empty((P, 2 * F), bf)
        xk[:, 0:F] = lb[sl].reshape(P, F)
        xk[:, F : 2 * F] = tb[sl].reshape(P, F)
        in_maps.append({"x": xk})
    return in_maps


def combine(outs: np.ndarray) -> np.ndarray:
    """All-reduce the [NCORES, P, 5] partials and apply the closed form."""
    tot = outs.astype(np.float64).sum(axis=(0, 1))
    g2, m1, g1, c, m2 = tot
    n_pos = c
    n_neg = float(N) - c
    sp1 = c - m1
    sp2 = c - 2.0 * m1 + m2
    sn1 = g1 - m1
    sn2 = g2 - m2
    loss = (n_neg * sp2 + 2.0 * sp1 * sn1 + n_pos * sn2) / (n_pos * n_neg)
    return np.array(loss, dtype=np.float32)


def kernel(logits: np.ndarray, targets: np.ndarray, **run_kwargs):
    nc = _get_nc()
    res = bass_utils.run_bass_kernel_spmd(
        nc, make_in_maps(logits, targets), core_ids=list(range(NCORES)), **run_kwargs
    )
    outs = np.stack([np.asarray(r["o"]).reshape(P, 5) for r in res.results])
    out = combine(outs)
    _CACHE["last_results"] = res
    return out
